# revision 50
# baseline (speedup 1.0000x reference)
"""Trainium2 Bass kernel for BitLTIInjection (BitNet-style fake-quantized linear
+ LTI injection):

    A_eff = 0.99*tanh(A_raw)
    e_q   = per-token absmax int8 fake quant of e
    W_q   = absmean ternary fake quant of W
    out   = A_eff*h + e_q @ W_q.T + block_out

Strategy: data-parallel over B*T across 8 cores; W replicated.  The quantized
matmul runs in bf16 (all quantized values are small integers, so the bf16
matmul with fp32 PSUM accumulation is numerically exact); dequant scales are
folded into the PSUM->SBUF epilogue.  Rounding uses the f32 magic-number trick
(x + 1.5*2^23 - 1.5*2^23) which matches round-half-even of jnp.round.
"""

import numpy as np

import concourse.bass as bass
import concourse.mybir as mybir
import concourse.tile as tile
from concourse.tile_rust import add_dep_helper
from concourse.bass import ts
from concourse.bass_utils import run_bass_kernel_spmd

P = 128
MAGIC = 12582912.0  # 1.5 * 2**23: forces RNE-to-integer in f32
EPS = 1e-5
N_CORES = 8
F32 = mybir.dt.float32
BF16 = mybir.dt.bfloat16
MM_N = 512  # moving free dim per matmul (one PSUM bank of f32)
WS_BUFS = 3  # W f32 staging slots


def build_kernel_body(tc: tile.TileContext, io: dict, Tc: int, D: int, with_h: bool, n_cores: int = N_CORES):
    nc = tc.nc
    n_tb = Tc // P  # token blocks per core
    n_dc = D // P   # contraction chunks
    n_ob = D // MM_N  # output column blocks
    n_wt = D // P   # weight row tiles

    e_d = io["e"]
    bo_d = io["bo"]
    w_d = io["w"]
    out_d = io["out"]

    with (
        tc.tile_pool(name="wq", bufs=1) as wq_pool,
        tc.tile_pool(name="ws", bufs=WS_BUFS) as ws_pool,
        tc.tile_pool(name="wtn", bufs=2) as wtn_pool,
        tc.tile_pool(name="wts", bufs=2) as wts_pool,
        tc.tile_pool(name="scal", bufs=1) as scal_pool,
        tc.tile_pool(name="st", bufs=3) as st_pool,
        tc.tile_pool(name="ep", bufs=2) as ep_pool,
        tc.tile_pool(name="pp", bufs=2 * n_ob, space="PSUM") as pp_pool,
    ):
        # ones vectors for cross-partition sum / broadcast via PE
        ones_col = scal_pool.tile([P, 1], F32, tag="ones_col")
        nc.vector.memset(ones_col[:], 1.0)
        ones_row = scal_pool.tile([1, P], F32, tag="ones_row")
        nc.vector.memset(ones_row[:], 1.0)
        c127 = scal_pool.tile([P, 1], F32, tag="c127")
        nc.vector.memset(c127[:], 127.0)
        negmagic = scal_pool.tile([P, 1], F32, tag="negmagic")
        nc.vector.memset(negmagic[:], -MAGIC)
        posmagic = scal_pool.tile([P, 1], F32, tag="posmagic")
        nc.vector.memset(posmagic[:], MAGIC)

        # ---------------- W absmean pass (full local, loads kept hot) --------
        parts = scal_pool.tile([P, n_wt], F32, tag="parts")
        wf_tiles = {}
        for j in range(n_wt):
            wf = ws_pool.tile([P, D], F32, tag="wf32", name=f"wfm_{j}")
            nc.sync.dma_start(out=wf[:], in_=w_d[ts(j, P), :])
            nc.vector.tensor_reduce(
                out=parts[:, j : j + 1],
                in_=wf[:],
                axis=mybir.AxisListType.X,
                op=mybir.AluOpType.add,
                apply_absolute_value=True,
            )
            wf_tiles[j] = wf

        acc = scal_pool.tile([P, 1], F32, tag="acc")
        nc.vector.tensor_reduce(
            out=acc[:], in_=parts[:], axis=mybir.AxisListType.X,
            op=mybir.AluOpType.add,
        )
        # cross-partition sum + broadcast via tiny PE matmuls
        tot_ps = pp_pool.tile([P, MM_N], F32, tag="ps", name="tot_ps")
        nc.tensor.matmul(tot_ps[:1, :1], ones_col[:], acc[:])
        tot_sb = scal_pool.tile([1, 1], F32, tag="tot_sb")
        nc.vector.tensor_copy(out=tot_sb[:], in_=tot_ps[:1, :1])
        asum_ps = pp_pool.tile([P, MM_N], F32, tag="ps", name="asum_ps")
        nc.tensor.matmul(asum_ps[:, :1], ones_row[:], tot_sb[:])
        allsum = scal_pool.tile([P, 1], F32, tag="allsum")
        nc.vector.tensor_copy(out=allsum[:], in_=asum_ps[:, :1])
        # m = max(mean_abs, EPS); s_w = 1/m ; deqm = m/127
        m_t = scal_pool.tile([P, 1], F32, tag="m_t")
        nc.vector.tensor_scalar(
            out=m_t[:], in0=allsum[:], scalar1=1.0 / (D * D), scalar2=EPS,
            op0=mybir.AluOpType.mult, op1=mybir.AluOpType.max,
        )
        # s_w = 1/m via reciprocal + one Newton step: r1 = r0*(2 - m*r0)
        r0w = scal_pool.tile([P, 1], F32, tag="r0w")
        nc.vector.reciprocal(r0w[:], m_t[:])
        t1w = scal_pool.tile([P, 1], F32, tag="t1w")
        nc.vector.scalar_tensor_tensor(
            out=t1w[:], in0=m_t[:], scalar=-1.0, in1=r0w[:],
            op0=mybir.AluOpType.mult, op1=mybir.AluOpType.mult,
        )
        nc.vector.tensor_scalar_add(t1w[:], t1w[:], 2.0)
        s_w = scal_pool.tile([P, 1], F32, tag="s_w")
        nc.vector.tensor_scalar_mul(s_w[:], r0w[:], t1w[:])
        deqm = scal_pool.tile([P, 1], F32, tag="deqm")
        nc.vector.tensor_scalar_mul(deqm[:], m_t[:], 1.0 / 127.0)

        # ---------------- W ternarize + transpose ----------------
        wqt = wq_pool.tile([P, n_dc, D], BF16, tag="wqt")
        n_res = WS_BUFS  # last n_res mean tiles are still pool-resident
        tern_order = list(range(n_wt - n_res, n_wt)) + list(range(n_wt - n_res))
        for j in tern_order:
            if j >= n_wt - n_res:
                wf = wf_tiles[j]
            else:
                wf = ws_pool.tile([P, D], F32, tag="wf32", name=f"wft_{j}")
                nc.sync.dma_start(out=wf[:], in_=w_d[ts(j, P), :])
            # tA = w*s_w + MAGIC on ACT (scale+bias stages; rounds to int RNE)
            tA = wtn_pool.tile([P, D], F32, tag="wtA")
            nc.scalar.activation(
                tA[:], wf[:], mybir.ActivationFunctionType.Identity,
                bias=posmagic[:], scale=s_w[:],
            )
            # tB = tA - MAGIC -> bf16 (exact small ints; ACT Identity bias-add)
            tB = wtn_pool.tile([P, D], BF16, tag="wtB")
            last_wpb = nc.scalar.activation(
                tB[:], tA[:], mybir.ActivationFunctionType.Identity,
                bias=negmagic[:], scale=1.0,
            )
            # transpose [o=128, d=D] -> [d0=128, dc, o=128]
            wts = wts_pool.tile([P, n_dc, P], BF16, tag="wts")
            last_wtr = nc.scalar.dma_start_transpose(out=wts[:], in_=tB[:])
            # fused clip(-1,1) + copy into the resident transposed weights
            last_wcc = nc.vector.tensor_scalar(
                out=wqt[:, :, ts(j, P)], in0=wts[:], scalar1=1.0, scalar2=-1.0,
                op0=mybir.AluOpType.min, op1=mybir.AluOpType.max,
            )

        # ---------------- A_eff (only if nonzero A_raw) ----------------
        if with_h:
            a_d = io["a_raw"]
            h_d = io["h"]
            a1 = scal_pool.tile([1, D], F32, tag="a1")
            nc.sync.dma_start(out=a1[:], in_=a_d[:, :])
            aeff = scal_pool.tile([P, D], F32, tag="aeff")
            for ob in range(n_ob):
                ab_ps = pp_pool.tile(
                    [P, MM_N], F32, tag="ps", name=f"ab_ps{ob}"
                )
                nc.tensor.matmul(ab_ps[:], ones_row[:], a1[:, ts(ob, MM_N)])
                nc.vector.tensor_copy(
                    out=aeff[:, ts(ob, MM_N)], in_=ab_ps[:]
                )
            nc.scalar.activation(
                aeff[:], aeff[:], mybir.ActivationFunctionType.Tanh
            )
            nc.vector.tensor_scalar_mul(aeff[:], aeff[:], 0.99)

        # ---------------- main token-block pipeline ----------------
        for i in range(n_tb):
            ef = ep_pool.tile([P, D], F32, tag="ef")
            nc.sync.dma_start(out=ef[:], in_=e_d[ts(i, P), :])
            bo_t = ep_pool.tile([P, D], F32, tag="bo")
            nc.gpsimd.dma_start(out=bo_t[:], in_=bo_d[ts(i, P), :])

            rmax = st_pool.tile([P, 1], F32, tag="rmax")
            nc.vector.tensor_reduce(
                out=rmax[:], in_=ef[:], axis=mybir.AxisListType.X,
                op=mybir.AluOpType.max, apply_absolute_value=True,
            )
            rm_c = st_pool.tile([P, 1], F32, tag="rm_c")
            nc.vector.tensor_scalar_max(rm_c[:], rmax[:], EPS)
            # scale = 127/rm_c with one Newton step on the reciprocal
            r0 = st_pool.tile([P, 1], F32, tag="r0")
            nc.vector.reciprocal(r0[:], rm_c[:])
            t1 = st_pool.tile([P, 1], F32, tag="t1s")
            nc.vector.scalar_tensor_tensor(
                out=t1[:], in0=rm_c[:], scalar=-1.0, in1=r0[:],
                op0=mybir.AluOpType.mult, op1=mybir.AluOpType.mult,
            )
            nc.vector.tensor_scalar_add(t1[:], t1[:], 2.0)
            nc.vector.tensor_scalar_mul(r0[:], r0[:], t1[:])
            scale = st_pool.tile([P, 1], F32, tag="scale")
            nc.vector.tensor_scalar_mul(scale[:], r0[:], 127.0)
            deq = st_pool.tile([P, 1], F32, tag="deq")
            nc.vector.tensor_scalar_mul(deq[:], rm_c[:], deqm[:])

            # quantize: ef = e*scale + MAGIC ; qb = ef - MAGIC (bf16)
            nc.vector.tensor_scalar(
                out=ef[:], in0=ef[:], scalar1=scale[:], scalar2=MAGIC,
                op0=mybir.AluOpType.mult, op1=mybir.AluOpType.add,
            )
            qb = ep_pool.tile([P, D], BF16, tag="qb")
            epb = nc.scalar.activation(
                qb[:], ef[:], mybir.ActivationFunctionType.Identity,
                bias=negmagic[:], scale=1.0,
            )
            if i == 2:
                # keep the W ACT chain (passA/passB) ahead of the e-quant
                # ACT ops so weight prep is never paced by the e pipeline
                add_dep_helper(
                    last_wpb.ins, epb.ins, sync=False,
                    reason="W ACT chain before e passB",
                )

            eT = ep_pool.tile([P, n_dc, P], BF16, tag="eT", bufs=3)
            etr = nc.scalar.dma_start_transpose(out=eT[:], in_=qb[:])
            if i == 2:
                # e transposes are gated on PE progress (eT slot reuse);
                # keep them behind the whole W transpose chain on ACT
                add_dep_helper(
                    last_wtr.ins, etr.ins, sync=False,
                    reason="W transposes before e transposes",
                )

            for ob in range(n_ob):
                ps = pp_pool.tile([P, MM_N], F32, tag="ps", name=f"ps{i}_{ob}")
                for d in range(n_dc):
                    nc.tensor.matmul(
                        ps[:],
                        eT[:, d, :],
                        wqt[:, d, ts(ob, MM_N)],
                        start=(d == 0),
                        stop=(d == n_dc - 1),
                    )
                # bo = psum * deq + block_out   (fused dequant + add, in place)
                epi = nc.vector.scalar_tensor_tensor(
                    out=bo_t[:, ts(ob, MM_N)],
                    in0=ps[:],
                    scalar=deq[:],
                    in1=bo_t[:, ts(ob, MM_N)],
                    op0=mybir.AluOpType.mult,
                    op1=mybir.AluOpType.add,
                )
                if i == 0 and ob == 0:
                    # keep the whole W DVE chain ahead of the (PE-gated)
                    # epilogues in the DVE stream so weight prep is never
                    # blocked behind matmul progress
                    add_dep_helper(
                        last_wcc.ins, epi.ins, sync=False,
                        reason="W-prep before epilogues on DVE",
                    )
            if with_h:
                hf = ep_pool.tile([P, D], F32, tag="hf")
                nc.gpsimd.dma_start(out=hf[:], in_=h_d[ts(i, P), :])
                nc.vector.tensor_tensor(
                    out=hf[:], in0=hf[:], in1=aeff[:], op=mybir.AluOpType.mult
                )
                nc.vector.tensor_tensor(
                    out=bo_t[:], in0=bo_t[:], in1=hf[:], op=mybir.AluOpType.add
                )
            nc.gpsimd.dma_start(out=out_d[ts(i, P), :], in_=bo_t[:])


def legalize_waits(nc):
    """Walrus in this container encodes at most ONE sync wait per ISA
    instruction (the 64B Events field) and refuses to split.  Rewrite any
    instruction carrying N>1 waits into N-1 single-wait NOP carrier
    instructions on the same engine placed immediately before it, keeping one
    wait on the original.  Waits are monotonic sem>=v conditions, so splitting
    preserves semantics exactly."""
    import bass_rust

    eng_map = {
        mybir.EngineType.SP: nc.sync,
        mybir.EngineType.DVE: nc.vector,
        mybir.EngineType.Activation: nc.scalar,
        mybir.EngineType.PE: nc.tensor,
        mybir.EngineType.Pool: nc.gpsimd,
    }
    for f in nc.m.functions:
        for blk in f.blocks:
            insts = list(blk.instructions)
            if not any(
                i.sync_info is not None and len(i.sync_info.on_wait) > 1
                for i in insts
            ):
                continue
            carriers = {}  # target inst name -> list of carrier insts
            for inst in insts:
                si = inst.sync_info
                if si is None or len(si.on_wait) <= 1:
                    continue
                waits = list(si.on_wait)
                cs = []
                for w in waits[:-1]:
                    bi = eng_map[inst.engine].nop(nofuse=True)
                    nop_inst = bi.ins
                    nop_inst.sync_info = bass_rust.SyncInfo(
                        on_wait=[w], on_update=[]
                    )
                    cs.append(nop_inst)
                carriers[inst.name] = cs
                inst.sync_info = bass_rust.SyncInfo(
                    on_wait=[waits[-1]], on_update=list(si.on_update)
                )
            # nops were appended to the current bb; remove them from wherever
            # they landed and splice before their targets.
            carrier_names = {c.name for cs in carriers.values() for c in cs}
            for f2 in nc.m.functions:
                for blk2 in f2.blocks:
                    cur = list(blk2.instructions)
                    if any(i.name in carrier_names for i in cur):
                        blk2.instructions = [
                            i for i in cur if i.name not in carrier_names
                        ]
            new_list = []
            for inst in blk.instructions:
                for c in carriers.get(inst.name, ()):
                    new_list.append(c)
                new_list.append(inst)
            blk.instructions = new_list


def build_nc(Tc: int, D: int, with_h: bool, n_cores: int = N_CORES):
    nc = bass.Bass("TRN2", target_bir_lowering=False, debug=False)
    io = {
        "e": nc.declare_dram_parameter("e", [Tc, D], F32, isOutput=False)[:],
        "bo": nc.declare_dram_parameter("bo", [Tc, D], F32, isOutput=False)[:],
        "w": nc.declare_dram_parameter("w", [D, D], F32, isOutput=False)[:],
    }
    if with_h:
        io["h"] = nc.declare_dram_parameter("h", [Tc, D], F32, isOutput=False)[:]
        io["a_raw"] = nc.declare_dram_parameter("a_raw", [1, D], F32, isOutput=False)[:]
    io["out"] = nc.declare_dram_parameter("out", [Tc, D], F32, isOutput=True)[:]
    with tile.TileContext(nc) as tc:
        build_kernel_body(tc, io, Tc, D, with_h, n_cores=n_cores)
    legalize_waits(nc)
    return nc


_NC_CACHE: dict = {}


def _get_nc(Tc: int, D: int, with_h: bool):
    key = (Tc, D, with_h)
    if key not in _NC_CACHE:
        _NC_CACHE[key] = build_nc(Tc, D, with_h)
    return _NC_CACHE[key]


def kernel(h, e, block_out, A_raw, W, _trace=False, _trace_kwargs=None):
    Bb, Tt, D = e.shape
    rows = Bb * Tt
    Tc = rows // N_CORES
    e2 = e.reshape(rows, D)
    bo2 = block_out.reshape(rows, D)
    h2 = h.reshape(rows, D)
    with_h = bool(np.any(A_raw))

    nc = _get_nc(Tc, D, with_h)
    in_maps = []
    for c in range(N_CORES):
        sl = slice(c * Tc, (c + 1) * Tc)
        m = {
            "e": np.ascontiguousarray(e2[sl]),
            "bo": np.ascontiguousarray(bo2[sl]),
            "w": np.ascontiguousarray(W),
        }
        if with_h:
            m["h"] = np.ascontiguousarray(h2[sl])
            m["a_raw"] = np.ascontiguousarray(A_raw.reshape(1, D))
        in_maps.append(m)

    res = run_bass_kernel_spmd(
        nc, in_maps, list(range(N_CORES)), trace=_trace,
        **(_trace_kwargs or {}),
    )
    out = np.concatenate([res.results[c]["out"] for c in range(N_CORES)], axis=0)
    if _trace:
        return out.reshape(Bb, Tt, D), res
    return out.reshape(Bb, Tt, D)
